# revision 1
# baseline (speedup 1.0000x reference)
"""Trainium2 Bass kernel for DualTimeConstantHighPassMixAdaptation.

Math (reference):
    xr = relu(x)
    Mf[t] = (1-mu_f)*Mf[t-1] + mu_f*xr[t],  Mf[0] = xr[0]   (same for Ms)
    M  = ma*Mf + (1-ma)*Ms,   ma = sigmoid(mix_weight_adapt)
    out = xr/(eps+M) + mh*(xr - M),         mh = sigmoid(mix_weight_hp)

Kernel formulation (scales folded so the tail is cheap):
    Q = mh*ma*mu_f ; R = mh*(1-ma)*mu_s ; E = mh*eps
    xf = relu(Q*x)                               [ACT]
    xs = xf*(R/Q) + E*mu_s                       [ACT]
    Pf = scan(af, xf)   -> mh*ma*Mf              [DVE]
    Ps = scan(as, xs)   -> mh*((1-ma)*Ms + eps)  [DVE]  (eps is the scan fixed point)
    S  = Pf + Ps        -> mh*(M + eps)          [DVE/GPSIMD]
    q  = 1/S  (fast approx)                      [DVE]
    r2 = q*(mh/Q) + (mh/Q)                       [ACT]
    t1 = r2 * xf        -> (1/(M+eps) + mh)*xr   [GPSIMD/DVE]
    out = t1 - S        (drops +mh*eps ~ 5.7e-7) [GPSIMD/DVE]

Sharding: core b <- batch b (64 channel lanes). In-core the 64 lanes are
split into two time halves -> 128 partitions x 32000 samples. Half-1 rows
get their scan initial state from a prepass that re-scans the last
W samples of half-0 (EMA decay bounds the truncation error ~a^W).
"""

import sys

for _p in ("/opt/trn_rl_repo", "/root/.axon_site/_ro/trn_rl_repo"):
    if _p not in sys.path:
        sys.path.insert(0, _p)

from contextlib import ExitStack

import numpy as np

import concourse.bacc as bacc
import concourse.tile as tile
from concourse import mybir
from concourse.bass_utils import run_bass_kernel_spmd

_dt = mybir.dt.float32
_A = mybir.AluOpType
_Act = mybir.ActivationFunctionType

# Problem geometry (hardcoded per spec).
B, C, T = 8, 64, 64000
HALF = T // 2          # 32000
FT = 2000              # main-loop chunk columns
NCHUNK = HALF // FT    # 20
W_SLOW = 16384         # slow-EMA prepass window
W_FAST = 1024          # fast-EMA prepass window
PRE_FT = 1024
NPRE = W_SLOW // PRE_FT
EPS = np.float32(1e-6)

# engine split for the two flexible tensor-tensor ops (columns to DVE)
SPLIT_T1 = 0     # t1 mult: 0 -> all GPSIMD
SPLIT_OUT = 0    # out sub: 0 -> all GPSIMD
SPLIT_S = FT     # S add: columns to DVE, rest GPSIMD
BUFS = 3


def _f32(v) -> np.float32:
    return np.float32(np.asarray(v).reshape(()))


def _tt_split(nc, split, out, in0, in1, op):
    """Column-split a tensor-tensor op between DVE ([:split]) and GPSIMD."""
    n = out.shape[1]
    if 0 < split < n:
        nc.vector.tensor_tensor(out[:, :split], in0[:, :split], in1[:, :split], op=op)
        nc.gpsimd.tensor_tensor(out[:, split:], in0[:, split:], in1[:, split:], op=op)
    elif split >= n:
        nc.vector.tensor_tensor(out[:], in0[:], in1[:], op=op)
    else:
        nc.gpsimd.tensor_tensor(out[:], in0[:], in1[:], op=op)


def _build(consts: dict, reps: int = 0):
    af = float(consts["af"]); as_ = float(consts["as"])
    Q = float(consts["Q"]); R = float(consts["R"])
    E = float(consts["E"]); mu_s = float(consts["mu_s"])
    mh_ma = float(consts["mh_ma"]); mh_1ma = float(consts["mh_1ma"])
    r2_sc = float(consts["r2_sc"])

    nc = bacc.Bacc("TRN2", target_bir_lowering=False, debug=False, num_devices=B)
    x_d = nc.dram_tensor("x", [C, T], _dt, kind="ExternalInput")
    y_d = nc.dram_tensor("y", [C, T], _dt, kind="ExternalOutput")

    # [half, lane, t] view; DMA walks (half, lane) as the 128 partitions.
    xh = x_d.ap().rearrange("l (h t) -> h l t", h=2)
    yh = y_d.ap().rearrange("l (h t) -> h l t", h=2)

    with tile.TileContext(nc) as tc, ExitStack() as ctx:
        cpool = ctx.enter_context(tc.tile_pool(name="consts", bufs=1))
        # col 0: af, col 1: as, col 2: init_f, col 3: init_s
        cons = cpool.tile([128, 4], _dt, tag="cons")
        nc.vector.memset(cons[:, 0:1], af)
        nc.vector.memset(cons[:, 1:2], as_)
        def af_b(p, f):
            a = cons[:, 0:1][p, :]
            return a.broadcast_to((a.shape[0], f))

        def as_b(p, f):
            a = cons[:, 1:2][p, :]
            return a.broadcast_to((a.shape[0], f))
        init_f = cons[:, 2:3]
        init_s = cons[:, 3:4]

        mpool = ctx.enter_context(tc.tile_pool(name="main", bufs=BUFS))

        def body(_iv=None):
            # ---- prepass: half-1 scan initials from half-0 tail ----
            pre_prev = None
            pre_f_end = None
            for k in range(NPRE):
                lo = HALF - W_SLOW + k * PRE_FT
                xp = mpool.tile([128, FT], _dt, tag="x")
                nc.sync.dma_start(xp[64:128, :PRE_FT], x_d.ap()[:, lo:lo + PRE_FT])
                if k == NPRE - 1:
                    fin = mpool.tile([128, FT], _dt, tag="xf")
                    nc.scalar.activation(
                        fin[64:128, :W_FAST],
                        xp[64:128, PRE_FT - W_FAST:PRE_FT],
                        _Act.Relu, scale=Q)
                nc.scalar.activation(xp[64:128, :PRE_FT], xp[64:128, :PRE_FT],
                                     _Act.Relu, scale=R)
                po = mpool.tile([128, FT], _dt, tag="ps")
                ini = 0.0 if pre_prev is None else pre_prev[64:128, PRE_FT - 1:PRE_FT]
                nc.vector.tensor_tensor_scan(
                    po[64:128, :PRE_FT], as_b(slice(64, 128), PRE_FT),
                    xp[64:128, :PRE_FT], ini, _A.mult, _A.add)
                pre_prev = po
                if k == NPRE - 1:
                    fo = mpool.tile([128, FT], _dt, tag="pf")
                    nc.vector.tensor_tensor_scan(
                        fo[64:128, :W_FAST], af_b(slice(64, 128), W_FAST),
                        fin[64:128, :W_FAST], 0.0, _A.mult, _A.add)
                    pre_f_end = fo

            nc.scalar.copy(init_f[64:128, :], pre_f_end[64:128, W_FAST - 1:W_FAST])
            nc.scalar.copy(init_s[64:128, :], pre_prev[64:128, PRE_FT - 1:PRE_FT])

            # ---- main streaming loop ----
            prev_f = None
            prev_s = None
            for j in range(NCHUNK):
                sl = slice(j * FT, (j + 1) * FT)
                xt = mpool.tile([128, FT], _dt, tag="x")
                nc.sync.dma_start(xt[:], xh[:, :, sl])

                if j == 0:
                    nc.scalar.activation(init_f[0:64, :], xt[0:64, 0:1],
                                         _Act.Relu, scale=mh_ma)
                    nc.scalar.activation(init_s[0:64, :], xt[0:64, 0:1],
                                         _Act.Relu, scale=mh_1ma)

                xf = mpool.tile([128, FT], _dt, tag="xf")
                nc.scalar.activation(xf[:], xt[:], _Act.Relu, scale=Q)
                xs = xt  # in-place relu (proven safe in prepass)
                nc.scalar.activation(xs[:], xt[:], _Act.Relu, scale=R)

                pf = mpool.tile([128, FT], _dt, tag="pf")
                ini_f = init_f if j == 0 else prev_f[:, FT - 1:FT]
                nc.vector.tensor_tensor_scan(pf[:], af_b(slice(None), FT),
                                             xf[:], ini_f, _A.mult, _A.add)
                ps = mpool.tile([128, FT], _dt, tag="ps")
                ini_s = init_s if j == 0 else prev_s[:, FT - 1:FT]
                nc.vector.tensor_tensor_scan(ps[:], as_b(slice(None), FT),
                                             xs[:], ini_s, _A.mult, _A.add)
                prev_f, prev_s = pf, ps

                # S = (Pf + E) + Ps  — the mh*eps constant rides the stt op
                s_t = mpool.tile([128, FT], _dt, tag="s")
                nc.vector.scalar_tensor_tensor(
                    s_t[:], pf[:], E, ps[:], _A.add, _A.add)

                q_t = mpool.tile([128, FT], _dt, tag="q")
                nc.vector.reciprocal_approx_fast(q_t[:], s_t[:])

                r2 = mpool.tile([128, FT], _dt, tag="r2")
                nc.scalar.activation(r2[:], q_t[:], _Act.Copy,
                                     scale=r2_sc, bias=r2_sc)

                t1 = q_t  # reuse (q dead after r2)
                _tt_split(nc, SPLIT_T1, t1, r2, xf, _A.mult)

                o_t = r2  # reuse (r2 dead after t1)
                _tt_split(nc, SPLIT_OUT, o_t, t1, s_t, _A.subtract)

                nc.gpsimd.dma_start(yh[:, :, sl], o_t[:])

        if reps > 0:
            for _ in range(reps):
                body()
        else:
            body()

    nc.compile()
    return nc


_CACHE: dict = {}


def _get_nc(consts: dict, reps: int = 0):
    key = (tuple(sorted(consts.items())), reps, SPLIT_T1, SPLIT_OUT, SPLIT_S, BUFS)
    if key not in _CACHE:
        _CACHE[key] = _build(consts, reps)
    return _CACHE[key]


def _consts_from_inputs(mu_fast, mu_slow, mix_weight_adapt, mix_weight_hp) -> dict:
    mu_f = _f32(mu_fast)
    mu_s = _f32(mu_slow)
    one = np.float32(1.0)
    ma = np.float32(one / (one + np.exp(np.float32(-mix_weight_adapt))))
    mh = np.float32(one / (one + np.exp(np.float32(-mix_weight_hp))))
    af = one - mu_f
    as_ = one - mu_s
    Q = np.float32(mh * ma * mu_f)
    R = np.float32(mh * (one - ma) * mu_s)
    E = np.float32(mh * EPS)
    d = dict(
        af=float(af),
        Q=float(Q), R=float(R), E=float(E), mu_s=float(mu_s),
        mh_ma=float(np.float32(mh * ma)), mh_1ma=float(np.float32(mh * (one - ma))),
        r2_sc=float(np.float32(mh / Q)),
    )
    d["as"] = float(as_)
    return d


def kernel(x, mu_fast, mu_slow, mix_weight_adapt, mix_weight_hp):
    x = np.asarray(x, dtype=np.float32)
    assert x.shape == (B, C, T), x.shape
    consts = _consts_from_inputs(mu_fast, mu_slow, mix_weight_adapt, mix_weight_hp)
    nc = _get_nc(consts)
    in_maps = [{"x": np.ascontiguousarray(x[b])} for b in range(B)]
    res = run_bass_kernel_spmd(nc, in_maps, core_ids=list(range(B)))
    return np.stack([res.results[b]["y"] for b in range(B)], axis=0)


if __name__ == "__main__":
    rng = np.random.default_rng(0)
    import math
    FS = 16000.0
    x = rng.standard_normal((B, C, T), dtype=np.float32)
    out = kernel(
        x,
        np.float32(1.0 - math.exp(-1.0 / (FS * 2.0 / 1000.0))),
        np.float32(1.0 - math.exp(-1.0 / (FS * 60.0 / 1000.0))),
        np.float32(0.5),
        np.float32(0.3),
    )
    print(out.shape, out.dtype, np.isfinite(out).all())



# revision 2
# speedup vs baseline: 1.0391x; 1.0391x over previous
"""Trainium2 Bass kernel for DualTimeConstantHighPassMixAdaptation.

Math (reference):
    xr = relu(x)
    Mf[t] = (1-mu_f)*Mf[t-1] + mu_f*xr[t],  Mf[0] = xr[0]   (same for Ms)
    M  = ma*Mf + (1-ma)*Ms,   ma = sigmoid(mix_weight_adapt)
    out = xr/(eps+M) + mh*(xr - M),         mh = sigmoid(mix_weight_hp)

Kernel formulation (scales folded so the tail is cheap):
    Q = mh*ma*mu_f ; R = mh*(1-ma)*mu_s ; E = mh*eps
    xf = relu(Q*x)                               [ACT]
    xs = xf*(R/Q) + E*mu_s                       [ACT]
    Pf = scan(af, xf)   -> mh*ma*Mf              [DVE]
    Ps = scan(as, xs)   -> mh*((1-ma)*Ms + eps)  [DVE]  (eps is the scan fixed point)
    S  = Pf + Ps        -> mh*(M + eps)          [DVE/GPSIMD]
    q  = 1/S  (fast approx)                      [DVE]
    r2 = q*(mh/Q) + (mh/Q)                       [ACT]
    t1 = r2 * xf        -> (1/(M+eps) + mh)*xr   [GPSIMD/DVE]
    out = t1 - S        (drops +mh*eps ~ 5.7e-7) [GPSIMD/DVE]

Sharding: core b <- batch b (64 channel lanes). In-core the 64 lanes are
split into two time halves -> 128 partitions x 32000 samples. Half-1 rows
get their scan initial state from a prepass that re-scans the last
W samples of half-0 (EMA decay bounds the truncation error ~a^W).
"""

import sys

for _p in ("/opt/trn_rl_repo", "/root/.axon_site/_ro/trn_rl_repo"):
    if _p not in sys.path:
        sys.path.insert(0, _p)

from contextlib import ExitStack

import numpy as np

import concourse.bacc as bacc
import concourse.tile as tile
from concourse import mybir
from concourse.bass_utils import run_bass_kernel_spmd

_dt = mybir.dt.float32
_A = mybir.AluOpType
_Act = mybir.ActivationFunctionType

# Problem geometry (hardcoded per spec).
B, C, T = 8, 64, 64000
HALF = T // 2          # 32000
FT = 2000              # main-loop chunk columns
NCHUNK = HALF // FT    # 20
W_SLOW = 16384         # slow-EMA prepass window
W_FAST = 1024          # fast-EMA prepass window
PRE_FT = 1024
NPRE = W_SLOW // PRE_FT
EPS = np.float32(1e-6)

# engine split for the two flexible tensor-tensor ops (columns to DVE)
SPLIT_T1 = 0     # t1 mult: 0 -> all GPSIMD
SPLIT_OUT = 0    # out sub: 0 -> all GPSIMD
SPLIT_S = FT     # S add: columns to DVE, rest GPSIMD
BUFS = 3


def _f32(v) -> np.float32:
    return np.float32(np.asarray(v).reshape(()))


def _tt_split(nc, split, out, in0, in1, op):
    """Column-split a tensor-tensor op between DVE ([:split]) and GPSIMD."""
    n = out.shape[1]
    if 0 < split < n:
        nc.vector.tensor_tensor(out[:, :split], in0[:, :split], in1[:, :split], op=op)
        nc.gpsimd.tensor_tensor(out[:, split:], in0[:, split:], in1[:, split:], op=op)
    elif split >= n:
        nc.vector.tensor_tensor(out[:], in0[:], in1[:], op=op)
    else:
        nc.gpsimd.tensor_tensor(out[:], in0[:], in1[:], op=op)


def _build(consts: dict, reps: int = 0):
    af = float(consts["af"]); as_ = float(consts["as"])
    Q = float(consts["Q"]); R = float(consts["R"])
    E = float(consts["E"]); mu_s = float(consts["mu_s"])
    mh_ma = float(consts["mh_ma"]); mh_1ma = float(consts["mh_1ma"])
    r2_sc = float(consts["r2_sc"])

    nc = bacc.Bacc("TRN2", target_bir_lowering=False, debug=False, num_devices=B)
    x_d = nc.dram_tensor("x", [C, T], _dt, kind="ExternalInput")
    y_d = nc.dram_tensor("y", [C, T], _dt, kind="ExternalOutput")

    # [half, lane, t] view; DMA walks (half, lane) as the 128 partitions.
    xh = x_d.ap().rearrange("l (h t) -> h l t", h=2)
    yh = y_d.ap().rearrange("l (h t) -> h l t", h=2)

    with tile.TileContext(nc) as tc, ExitStack() as ctx:
        cpool = ctx.enter_context(tc.tile_pool(name="consts", bufs=1))
        # col 0: af, col 1: as, col 2: init_f, col 3: init_s
        cons = cpool.tile([128, 4], _dt, tag="cons")
        nc.vector.memset(cons[:, 0:1], af)
        nc.vector.memset(cons[:, 1:2], as_)
        def af_b(p, f):
            a = cons[:, 0:1][p, :]
            return a.broadcast_to((a.shape[0], f))

        def as_b(p, f):
            a = cons[:, 1:2][p, :]
            return a.broadcast_to((a.shape[0], f))
        init_f = cons[:, 2:3]
        init_s = cons[:, 3:4]

        mpool = ctx.enter_context(tc.tile_pool(name="main", bufs=BUFS))

        def body(_iv=None):
            # ---- prepass: half-1 scan initials from half-0 tail ----
            pre_prev = None
            pre_f_end = None
            for k in range(NPRE):
                lo = HALF - W_SLOW + k * PRE_FT
                xp = mpool.tile([128, FT], _dt, tag="x")
                nc.sync.dma_start(xp[64:128, :PRE_FT], x_d.ap()[:, lo:lo + PRE_FT])
                if k == NPRE - 1:
                    fin = mpool.tile([128, FT], _dt, tag="xf")
                    nc.scalar.activation(
                        fin[64:128, :W_FAST],
                        xp[64:128, PRE_FT - W_FAST:PRE_FT],
                        _Act.Relu, scale=Q)
                nc.scalar.activation(xp[64:128, :PRE_FT], xp[64:128, :PRE_FT],
                                     _Act.Relu, scale=R)
                po = mpool.tile([128, FT], _dt, tag="ps")
                ini = 0.0 if pre_prev is None else pre_prev[64:128, PRE_FT - 1:PRE_FT]
                nc.vector.tensor_tensor_scan(
                    po[64:128, :PRE_FT], as_b(slice(64, 128), PRE_FT),
                    xp[64:128, :PRE_FT], ini, _A.mult, _A.add)
                pre_prev = po
                if k == NPRE - 1:
                    fo = mpool.tile([128, FT], _dt, tag="pf")
                    nc.vector.tensor_tensor_scan(
                        fo[64:128, :W_FAST], af_b(slice(64, 128), W_FAST),
                        fin[64:128, :W_FAST], 0.0, _A.mult, _A.add)
                    pre_f_end = fo

            nc.scalar.copy(init_f[64:128, :], pre_f_end[64:128, W_FAST - 1:W_FAST])
            nc.scalar.copy(init_s[64:128, :], pre_prev[64:128, PRE_FT - 1:PRE_FT])

            # ---- main streaming loop ----
            prev_f = None
            prev_s = None
            for j in range(NCHUNK):
                sl = slice(j * FT, (j + 1) * FT)
                xt = mpool.tile([128, FT], _dt, tag="x")
                # Two 2D loads, not one 3D: a 3D HWDGE InstDMACopy lands on
                # only 2 SDMA engines (~52 GB/s); 2D programs spread across
                # all 16.
                nc.sync.dma_start(xt[0:64, :], x_d.ap()[:, sl])
                nc.sync.dma_start(
                    xt[64:128, :],
                    x_d.ap()[:, HALF + j * FT:HALF + (j + 1) * FT])

                if j == 0:
                    nc.scalar.activation(init_f[0:64, :], xt[0:64, 0:1],
                                         _Act.Relu, scale=mh_ma)
                    nc.scalar.activation(init_s[0:64, :], xt[0:64, 0:1],
                                         _Act.Relu, scale=mh_1ma)

                xf = mpool.tile([128, FT], _dt, tag="xf")
                nc.scalar.activation(xf[:], xt[:], _Act.Relu, scale=Q)
                xs = xt  # in-place relu (proven safe in prepass)
                nc.scalar.activation(xs[:], xt[:], _Act.Relu, scale=R)

                pf = mpool.tile([128, FT], _dt, tag="pf")
                ini_f = init_f if j == 0 else prev_f[:, FT - 1:FT]
                nc.vector.tensor_tensor_scan(pf[:], af_b(slice(None), FT),
                                             xf[:], ini_f, _A.mult, _A.add)
                ps = mpool.tile([128, FT], _dt, tag="ps")
                ini_s = init_s if j == 0 else prev_s[:, FT - 1:FT]
                nc.vector.tensor_tensor_scan(ps[:], as_b(slice(None), FT),
                                             xs[:], ini_s, _A.mult, _A.add)
                prev_f, prev_s = pf, ps

                # S = (Pf + E) + Ps  — the mh*eps constant rides the stt op
                s_t = mpool.tile([128, FT], _dt, tag="s")
                nc.vector.scalar_tensor_tensor(
                    s_t[:], pf[:], E, ps[:], _A.add, _A.add)

                q_t = mpool.tile([128, FT], _dt, tag="q")
                nc.vector.reciprocal_approx_fast(q_t[:], s_t[:])

                r2 = mpool.tile([128, FT], _dt, tag="r2")
                nc.scalar.activation(r2[:], q_t[:], _Act.Copy,
                                     scale=r2_sc, bias=r2_sc)

                t1 = q_t  # reuse (q dead after r2)
                _tt_split(nc, SPLIT_T1, t1, r2, xf, _A.mult)

                o_t = r2  # reuse (r2 dead after t1)
                _tt_split(nc, SPLIT_OUT, o_t, t1, s_t, _A.subtract)

                nc.gpsimd.dma_start(yh[:, :, sl], o_t[:])

        if reps > 0:
            for _ in range(reps):
                body()
        else:
            body()

    nc.compile()
    return nc


_CACHE: dict = {}


def _get_nc(consts: dict, reps: int = 0):
    key = (tuple(sorted(consts.items())), reps, SPLIT_T1, SPLIT_OUT, SPLIT_S, BUFS)
    if key not in _CACHE:
        _CACHE[key] = _build(consts, reps)
    return _CACHE[key]


def _consts_from_inputs(mu_fast, mu_slow, mix_weight_adapt, mix_weight_hp) -> dict:
    mu_f = _f32(mu_fast)
    mu_s = _f32(mu_slow)
    one = np.float32(1.0)
    ma = np.float32(one / (one + np.exp(np.float32(-mix_weight_adapt))))
    mh = np.float32(one / (one + np.exp(np.float32(-mix_weight_hp))))
    af = one - mu_f
    as_ = one - mu_s
    Q = np.float32(mh * ma * mu_f)
    R = np.float32(mh * (one - ma) * mu_s)
    E = np.float32(mh * EPS)
    d = dict(
        af=float(af),
        Q=float(Q), R=float(R), E=float(E), mu_s=float(mu_s),
        mh_ma=float(np.float32(mh * ma)), mh_1ma=float(np.float32(mh * (one - ma))),
        r2_sc=float(np.float32(mh / Q)),
    )
    d["as"] = float(as_)
    return d


def kernel(x, mu_fast, mu_slow, mix_weight_adapt, mix_weight_hp):
    x = np.asarray(x, dtype=np.float32)
    assert x.shape == (B, C, T), x.shape
    consts = _consts_from_inputs(mu_fast, mu_slow, mix_weight_adapt, mix_weight_hp)
    nc = _get_nc(consts)
    in_maps = [{"x": np.ascontiguousarray(x[b])} for b in range(B)]
    res = run_bass_kernel_spmd(nc, in_maps, core_ids=list(range(B)))
    return np.stack([res.results[b]["y"] for b in range(B)], axis=0)


if __name__ == "__main__":
    rng = np.random.default_rng(0)
    import math
    FS = 16000.0
    x = rng.standard_normal((B, C, T), dtype=np.float32)
    out = kernel(
        x,
        np.float32(1.0 - math.exp(-1.0 / (FS * 2.0 / 1000.0))),
        np.float32(1.0 - math.exp(-1.0 / (FS * 60.0 / 1000.0))),
        np.float32(0.5),
        np.float32(0.3),
    )
    print(out.shape, out.dtype, np.isfinite(out).all())



# revision 16
# speedup vs baseline: 1.6925x; 1.6287x over previous
"""Trainium2 Bass kernel for DualTimeConstantHighPassMixAdaptation.

Math (reference):
    xr = relu(x)
    Mf[t] = (1-mu_f)*Mf[t-1] + mu_f*xr[t],  Mf[0] = xr[0]   (same for Ms)
    M  = ma*Mf + (1-ma)*Ms,   ma = sigmoid(mix_weight_adapt)
    out = xr/(eps+M) + mh*(xr - M),         mh = sigmoid(mix_weight_hp)

Kernel formulation:
    Pf = mh*ma*Mf, Ps = mh*(1-ma)*Ms, S = Pf + Ps + mh*eps = mh*(M+eps)
    q  = 1/S ;  t1 = (q+1)*mh*xr ;  out = t1 - S   (drops +mh*eps ~ 5.7e-7)

Each EMA chunk is ONE custom DVE op (bubble-free, ~1 elem/cycle/lane),
using the exponential-ramp diagonalization:
    Pf[t] = af^t * (af*carry + sum_k (Q*af^-k) * relu(x[k]))
body = scan(ADD, relu(Src0)*Src1, init=C0*C1) * geo(C1), where Src1 is a
precomputed decay ramp (constants folded in) and geo is a per-element
geometric ramp from the subdim-step scan (pages of 1).

Sharding: core b <- batch b (64 channel lanes). In-core the 64 lanes are
split into two time halves -> 128 partitions x 32000 samples. Half-1 rows
get their scan initial state from a prepass that re-scans the last
W samples of half-0 (EMA decay bounds the truncation error ~a^W).

Input loads / output stores are pairs of 2D HWDGE DMAs (a 3D access
pattern lands on only 2 of 16 SDMA engines; 2D spreads across all 16).
"""

import sys

for _p in ("/opt/trn_rl_repo", "/root/.axon_site/_ro/trn_rl_repo"):
    if _p not in sys.path:
        sys.path.insert(0, _p)

from contextlib import ExitStack

import numpy as np

import concourse.bacc as bacc
import concourse.tile as tile
from concourse import mybir
from concourse.bass_utils import run_bass_kernel_spmd

from concourse import dve_ops
from concourse.dve_spec import (AluOp as _DAlu, Bin as _DBin, Scan as _DScan,
                                Spec as _DSpec, Src0 as _Src0, Src1 as _Src1,
                                C0 as _C0, C1 as _C1, C2 as _C2, Zero as _DZero,
                                One as _DOne, relu as _drelu, lower as _dlower,
                                _has_src1)
from concourse.dve_uop import DveOpSpec as _DveOpSpec

_dt = mybir.dt.float32
_A = mybir.AluOpType
_Act = mybir.ActivationFunctionType

# Problem geometry (hardcoded per spec).
B, C, T = 8, 64, 64000
HALF = T // 2          # 32000
FT = 2000              # chunk columns
NCHUNK = HALF // FT    # 20
W_SLOW = 8000          # slow-EMA prepass window (as^8000 ~ 2.4e-4)
NPRE = W_SLOW // FT    # 4
EPS = np.float32(1e-6)

BUFS = 3
# columns of each flexible op placed on DVE (rest on GPSIMD)
SPLIT_T1 = 300
SPLIT_OUT = 300


def _f32(v) -> np.float32:
    return np.float32(np.asarray(v).reshape(()))


# ---- custom DVE op: fused relu+EMA scan with ramp diagonalization ----

def _ema_ref(in0, in1, s0, s1, imm2):
    x = np.asarray(in0, np.float32)
    P = x.shape[0]
    xf = x.reshape(P, -1)
    Tn = xf.shape[1]
    r = np.asarray(in1, np.float32).reshape(P, -1)
    s0 = np.asarray(s0, np.float32).reshape(P, 1)
    u = np.maximum(xf, 0.0) * r
    u = s0 * np.float32(s1) + np.cumsum(u, axis=1, dtype=np.float32)
    g = np.float32(s1) ** np.arange(Tn, dtype=np.float32)
    return (u * g).astype(np.float32).reshape(x.shape)


def _register_dve_op(name, spec, subdim):
    for o in dve_ops.OPS:
        if o.name == name:
            return o
    opcode = dve_ops._CUSTOM_DVE_ROW_BASE + len(dve_ops.OPS)
    dve_ops._SUB_OPCODE_FOR_NAME[name] = opcode
    shas = {}
    for ver in ("v3", "v4"):
        uops = _dlower(spec, ver=ver)
        shas[ver] = _DveOpSpec(name=name, opcode=opcode, uops=uops,
                               rd1_en=_has_src1(spec)).sha(ver)
    op = dve_ops.DveOp(name, spec, subdim=subdim, uops_sha=shas)
    dve_ops.OPS.append(op)
    dve_ops.CUSTOM_DVE_SPECS[name] = spec
    return op


_geo = _DScan(_DAlu.MULTIPLY, _DZero, init=_DOne, _subdim_step=_C1)
_EMA_SPEC = _DSpec(
    body=_DScan(_DAlu.ADD, _drelu(_Src0) * _Src1,
                init=_DBin(_DAlu.MULTIPLY, _C0, _C1)) * _geo,
    reference=_ema_ref,
)
EMA_OP = _register_dve_op("RELU_EMA_RAMP_ANT", _EMA_SPEC, subdim=True)


def _tt_split(nc, split, out, in0, in1, op):
    n = out.shape[1]
    if 0 < split < n:
        nc.vector.tensor_tensor(out[:, :split], in0[:, :split], in1[:, :split], op=op)
        nc.gpsimd.tensor_tensor(out[:, split:], in0[:, split:], in1[:, split:], op=op)
    elif split >= n:
        nc.vector.tensor_tensor(out[:], in0[:], in1[:], op=op)
    else:
        nc.gpsimd.tensor_tensor(out[:], in0[:], in1[:], op=op)


def _build(consts: dict, reps: int = 0):
    af = float(consts["af"]); as_ = float(consts["as"])
    Q = float(consts["Q"]); R = float(consts["R"])
    E = float(consts["E"])
    mh = float(consts["mh"])
    mh_ma = float(consts["mh_ma"]); mh_1ma = float(consts["mh_1ma"])

    nc = bacc.Bacc("TRN2", target_bir_lowering=False, debug=False, num_devices=B)
    x_d = nc.dram_tensor("x", [C, T], _dt, kind="ExternalInput")
    y_d = nc.dram_tensor("y", [C, T], _dt, kind="ExternalOutput")

    def sub3(ap):
        return ap.rearrange("p (s n) -> p s n", n=1)

    with tile.TileContext(nc) as tc, ExitStack() as ctx:
        cpool = ctx.enter_context(tc.tile_pool(name="consts", bufs=1))
        # cols: 0 init_f, 1 init_s, 2 zero, 3 inv_af, 4 inv_as
        cons = cpool.tile([128, 8], _dt, tag="cons")
        nc.vector.memset(cons[:, 2:3], 0.0)
        nc.vector.memset(cons[:, 3:4], 1.0 / af)
        nc.vector.memset(cons[:, 4:5], 1.0 / as_)
        init_f = cons[:, 0:1]
        init_s = cons[:, 1:2]
        zero_b = cons[:, 2:3].broadcast_to((128, FT))
        inv_af_b = cons[:, 3:4].broadcast_to((128, FT))
        inv_as_b = cons[:, 4:5].broadcast_to((128, FT))

        rpool = ctx.enter_context(tc.tile_pool(name="ramps", bufs=1))
        rdf = rpool.tile([128, FT], _dt, tag="rdf")
        rds = rpool.tile([128, FT], _dt, tag="rds")
        # ramp[k] = c * a^-k  via stock scan: state = inv_a*state + 0,
        # init c*a  ->  out[0] = c, out[k] = c*a^-k
        nc.vector.tensor_tensor_scan(rdf[:], inv_af_b, zero_b,
                                     float(Q * af), _A.mult, _A.add)
        nc.vector.tensor_tensor_scan(rds[:], inv_as_b, zero_b,
                                     float(R * as_), _A.mult, _A.add)

        mpool = ctx.enter_context(tc.tile_pool(name="main", bufs=BUFS))

        def ema(out_ap, in_ap, ramp_ap, carry_ap, a):
            nc.vector._custom_dve(EMA_OP, out=out_ap, in0=sub3(in_ap),
                                  in1=ramp_ap, s0=carry_ap, s1=float(a))

        def body(_iv=None):
            # ---- prepass: half-1 scan initials from half-0 tail ----
            # A custom DVE op on a partial partition range is silently
            # skipped on HW — every ema() below must span all 128
            # partitions, so fill the unused half with zeros.
            pre_ps = None
            pre_pf = None
            for k in range(NPRE):
                lo = HALF - W_SLOW + k * FT
                xp = mpool.tile([128, FT], _dt, tag="x")
                nc.gpsimd.memset(xp[0:64, :], 0.0)
                nc.sync.dma_start(xp[64:128, :], x_d.ap()[:, lo:lo + FT])
                po = mpool.tile([128, FT], _dt, tag="ps")
                carry = cons[:, 2:3] if pre_ps is None \
                    else pre_ps[:, FT - 1:FT]
                ema(po[:], xp[:], rds[:], carry, as_)
                pre_ps = po
                if k == NPRE - 1:
                    fo = mpool.tile([128, FT], _dt, tag="pf")
                    ema(fo[:], xp[:], rdf[:], cons[:, 2:3], af)
                    pre_pf = fo
            nc.scalar.copy(init_f[64:128, :], pre_pf[64:128, FT - 1:FT])
            nc.scalar.copy(init_s[64:128, :], pre_ps[64:128, FT - 1:FT])

            # ---- main streaming loop ----
            prev_pf = None
            prev_ps = None
            for j in range(NCHUNK):
                sl = slice(j * FT, (j + 1) * FT)
                sh = slice(HALF + j * FT, HALF + (j + 1) * FT)
                xt = mpool.tile([128, FT], _dt, tag="x")
                # 2D loads spread over all 16 SDMA engines; 3D would not.
                nc.sync.dma_start(xt[0:64, :], x_d.ap()[:, sl])
                nc.sync.dma_start(xt[64:128, :], x_d.ap()[:, sh])

                if j == 0:
                    nc.scalar.activation(init_f[0:64, :], xt[0:64, 0:1],
                                         _Act.Relu, scale=mh_ma)
                    nc.scalar.activation(init_s[0:64, :], xt[0:64, 0:1],
                                         _Act.Relu, scale=mh_1ma)

                pf = mpool.tile([128, FT], _dt, tag="pf")
                cf = init_f if j == 0 else prev_pf[:, FT - 1:FT]
                ema(pf[:], xt[:], rdf[:], cf, af)
                ps = mpool.tile([128, FT], _dt, tag="ps")
                cs = init_s if j == 0 else prev_ps[:, FT - 1:FT]
                ema(ps[:], xt[:], rds[:], cs, as_)
                prev_pf, prev_ps = pf, ps

                xr = mpool.tile([128, FT], _dt, tag="xr")
                nc.scalar.activation(xr[:], xt[:], _Act.Relu, scale=mh)

                s3 = mpool.tile([128, FT], _dt, tag="s3")
                nc.vector.scalar_tensor_tensor(s3[:], pf[:], E, ps[:],
                                               _A.add, _A.add)

                q = mpool.tile([128, FT], _dt, tag="q")
                nc.vector.reciprocal_approx_fast(q[:], s3[:])

                r2 = q  # in-place: r2 = q + 1  [ACT]
                nc.scalar.activation(r2[:], q[:], _Act.Copy, bias=1.0)

                t1 = mpool.tile([128, FT], _dt, tag="t1")
                _tt_split(nc, SPLIT_T1, t1, r2, xr, _A.mult)

                o = xr  # xr dead after t1
                _tt_split(nc, SPLIT_OUT, o, t1, s3, _A.subtract)

                nc.gpsimd.dma_start(y_d.ap()[:, sl], o[0:64, :])
                nc.gpsimd.dma_start(y_d.ap()[:, sh], o[64:128, :])

        if reps > 0:
            for _ in range(reps):
                body()
        else:
            body()

    nc.compile()
    return nc


_CACHE: dict = {}


def _get_nc(consts: dict, reps: int = 0):
    key = (tuple(sorted(consts.items())), reps, BUFS, SPLIT_T1,
           SPLIT_OUT, FT, W_SLOW)
    if key not in _CACHE:
        _CACHE[key] = _build(consts, reps)
    return _CACHE[key]


def _consts_from_inputs(mu_fast, mu_slow, mix_weight_adapt, mix_weight_hp) -> dict:
    mu_f = _f32(mu_fast)
    mu_s = _f32(mu_slow)
    one = np.float32(1.0)
    ma = np.float32(one / (one + np.exp(np.float32(-mix_weight_adapt))))
    mh = np.float32(one / (one + np.exp(np.float32(-mix_weight_hp))))
    af = one - mu_f
    as_ = one - mu_s
    d = dict(
        af=float(af),
        Q=float(np.float32(mh * ma * mu_f)),
        R=float(np.float32(mh * (one - ma) * mu_s)),
        E=float(np.float32(mh * EPS)),
        mh=float(mh),
        mh_ma=float(np.float32(mh * ma)),
        mh_1ma=float(np.float32(mh * (one - ma))),
    )
    d["as"] = float(as_)
    return d


def kernel(x, mu_fast, mu_slow, mix_weight_adapt, mix_weight_hp):
    x = np.asarray(x, dtype=np.float32)
    assert x.shape == (B, C, T), x.shape
    consts = _consts_from_inputs(mu_fast, mu_slow, mix_weight_adapt, mix_weight_hp)
    nc = _get_nc(consts)
    in_maps = [{"x": np.ascontiguousarray(x[b])} for b in range(B)]
    res = run_bass_kernel_spmd(nc, in_maps, core_ids=list(range(B)))
    return np.stack([res.results[b]["y"] for b in range(B)], axis=0)


if __name__ == "__main__":
    rng = np.random.default_rng(0)
    import math
    FS = 16000.0
    x = rng.standard_normal((B, C, T), dtype=np.float32)
    out = kernel(
        x,
        np.float32(1.0 - math.exp(-1.0 / (FS * 2.0 / 1000.0))),
        np.float32(1.0 - math.exp(-1.0 / (FS * 60.0 / 1000.0))),
        np.float32(0.5),
        np.float32(0.3),
    )
    print(out.shape, out.dtype, np.isfinite(out).all())
